# revision 17
# baseline (speedup 1.0000x reference)
"""MoE layer (top-2 of 8 experts, selection shared across tokens) on 8 TRN2 cores.

Math (faithful to the reference):
    gates = softmax(x @ W_gate + b_gate)          [N, 8]
    idx0  = top-2 expert indices of token 0       [2]
    s     = per-token top-2 gate VALUES (desc)    [N, 2]
    out   = s0 * (x @ W[A] + b[A]) + s1 * (x @ W[B] + b[B])

Strategy: gating + top-2 is 0.2% of the FLOPs -> computed on host.  The two
active expert matmuls (275 GFLOP) are data-parallel sharded over tokens across
8 cores; expert weights are replicated.  Matmuls run in fp16 (values are small,
so fp16 range is safe and its 10-bit mantissa keeps rel-err ~3e-4),
accumulating fp32 in PSUM.

The PE stream (2048 MMs x 512 cols @ 2.4 GHz = 437 us) is the hard floor;
everything else is arranged to keep the PE saturated from ~11 us on:
  - x is resident in SBUF (loaded once, 8.4 MB fp16), W streams once per
    nb-block (no x re-streaming; total DMA-in 25 MB vs 50 MB).
  - DMA is coarse (0.5 MB tiles, >=4 KB per-partition descriptor runs):
    dma_start issue costs ~0.65 us, and SDMA round-robins between queues at
    PACKET granularity where a packet is one descriptor run -- small
    descriptors starve their queue.  x/W are host-PRE-PACKED per tile.
  - The cold fill is the binding constraint for the stream start: group 0
    needs xq0 + the full 4 MB W block, so those 5 MB are split evenly
    across both HWDGE queues in exactly the order the PE consumes them.
  - bias is replicated across partitions ON-CHIP (gpsimd partition_broadcast
    from an 8 KB row) so no constant competes with the fill; per-token
    scores (sC, 16 KB) trail the critical tiles.
  - outputs are written fp16 (host upcasts) on the HWDGE queues (the SWDGE
    end-of-kernel drain costs ~6.5 us; HWDGE drains instantly).
"""

import functools

import numpy as np

import concourse.bass as bass
import concourse.mybir as mybir
import concourse.tile as tile
from concourse import bacc
from concourse.bass_utils import run_bass_kernel_spmd

N_CORES = 8
N, D_IN, D_HID = 16384, 2048, 2048
NT = N // N_CORES            # tokens per core
KP = 128                     # contraction chunk = partition dim
KCH = D_IN // KP             # 16 K-chunks
KH = 8                       # K-chunks per x half-tile
KQ = 4                       # K-chunks per W quarter-tile
NB = 512                     # output column block (1 PSUM bank of fp32)
NBLK = D_HID // NB           # 4 output blocks
TQ = 256                     # tokens per resident x tile
NQ = NT // TQ                # 8 x tiles
MPQ = TQ // 128              # m-tiles per x tile
MG = NT // 128               # 16 token groups per core

F32 = mybir.dt.float32
FP16 = mybir.dt.float16

W_DT = FP16
X_DT = FP16
O_DT = FP16

# Filled by test harness inspection: last BassKernelResults from a run.
LAST_RESULT = None


@functools.lru_cache(maxsize=1)
def _build():
    nc = bacc.Bacc("TRN2", target_bir_lowering=False, debug=False)
    # Host-pre-packed: xQ[q, h] -> one x half-tile [128, KH, TQ] (4 KB/part);
    # wP[nb, e, j] -> one W quarter-tile [128, KQ, NB] (4 KB/part).
    xQ = nc.dram_tensor("xQ", [NQ, 2, 128, KH, TQ], X_DT, kind="ExternalInput")
    wP = nc.dram_tensor("wP", [NBLK, 2, 4, 128, KQ, NB], W_DT,
                        kind="ExternalInput")
    # bias row (replicated on-chip): b2[0, e, o] = b_sel[e, o]
    b2 = nc.dram_tensor("b2", [1, 2, D_HID], FP16, kind="ExternalInput")
    # per-token scores pre-arranged on host, partition-major:
    # sC[p, m, s] = top2_score[m*128 + p, s]
    sC = nc.dram_tensor("sC", [128, MG, 2], F32, kind="ExternalInput")
    out = nc.dram_tensor("out", [NT, D_HID], O_DT, kind="ExternalOutput")

    MULT = mybir.AluOpType.mult
    ADD = mybir.AluOpType.add

    with tile.TileContext(nc) as tc:
        with (
            tc.tile_pool(name="cst", bufs=1) as cst,
            tc.tile_pool(name="xp", bufs=1) as xp,
            tc.tile_pool(name="wp", bufs=2) as wp,
            tc.tile_pool(name="ep", bufs=2) as ep,
            tc.tile_pool(name="op", bufs=3) as op,
            tc.tile_pool(name="ps", bufs=4, space=bass.MemorySpace.PSUM) as ps,
        ):
            # bias: 8 KB row on the (otherwise idle) SWDGE queue, replicated
            # to all partitions by gpsimd via POOL ports -- zero SDMA traffic.
            b2_sb = cst.tile([1, 2, D_HID], FP16)
            nc.gpsimd.dma_start(b2_sb[:], b2[:])
            brep_sb = cst.tile([128, 2, D_HID], FP16)
            nc.gpsimd.partition_broadcast(brep_sb[:], b2_sb[:])

            hw_eng = [nc.sync, nc.scalar]

            def load_x(q, h, eng):
                t = xp.tile([KP, KH, TQ], X_DT, tag=f"xq{q}h{h}")
                eng.dma_start(t[:], xQ[q, h])
                return t

            def load_w(nb, e, j, eng):
                t = wp.tile([KP, KQ, NB], W_DT, tag=f"w{e}q{j}")
                eng.dma_start(t[:], wP[nb, e, j])
                return t

            # Cold fill: the 5 MB that group 0 consumes, split evenly across
            # the two queues in consumption order (pa's wa quarters on q10,
            # xq0 halves + pb's wb quarters on q1; last wb quarter evens the
            # byte split).  sC (16 KB) trails; first epilogue needs it ~30 us.
            x_t = [[None, None] for _ in range(NQ)]
            w_cur = {}
            x_t[0][0] = load_x(0, 0, nc.sync)
            w_cur[0, 0] = load_w(0, 0, 0, nc.scalar)
            x_t[0][1] = load_x(0, 1, nc.sync)
            w_cur[0, 1] = load_w(0, 0, 1, nc.scalar)
            w_cur[1, 0] = load_w(0, 1, 0, nc.sync)
            w_cur[0, 2] = load_w(0, 0, 2, nc.scalar)
            w_cur[1, 1] = load_w(0, 1, 1, nc.sync)
            w_cur[0, 3] = load_w(0, 0, 3, nc.scalar)
            w_cur[1, 2] = load_w(0, 1, 2, nc.sync)
            w_cur[1, 3] = load_w(0, 1, 3, nc.scalar)
            sC_sb = cst.tile([128, MG, 2], F32)
            nc.sync.dma_start(sC_sb[:], sC[:])
            for q in range(1, NQ):
                x_t[q][0] = load_x(q, 0, hw_eng[q % 2])
                x_t[q][1] = load_x(q, 1, hw_eng[(q + 1) % 2])

            for nb in range(NBLK):
                nb_sl = bass.ts(nb, NB)
                w_next = {} if nb + 1 < NBLK else None
                for q in range(NQ):
                    # W(nb+1) prefetch: one 0.5 MB quarter per q slice.
                    if w_next is not None:
                        e, j = q // 4, q % 4
                        w_next[e, j] = load_w(nb + 1, e, j, hw_eng[q % 2])
                    for mi in range(MPQ):
                        mg = q * MPQ + mi
                        m_sl = bass.ts(mi, 128)
                        pa = ps.tile([128, NB], F32, tag="pa")
                        pb = ps.tile([128, NB], F32, tag="pb")
                        pp = (pa, pb)
                        # expert-sequential, quarter-major: matches the cold
                        # fill's arrival order, and pa finishing early lets
                        # its epilogue ops overlap pb's accumulation.
                        for e in range(2):
                            for j in range(4):
                                for kk in range(KQ):
                                    nc.tensor.matmul(
                                        pp[e][:],
                                        x_t[q][j // 2][:, (j % 2) * KQ + kk, m_sl],
                                        w_cur[e, j][:, kk, :],
                                        start=(j == 0 and kk == 0),
                                        stop=(j == 3 and kk == KQ - 1),
                                    )
                        s0 = sC_sb[:, mg, 0:1]
                        s1 = sC_sb[:, mg, 1:2]
                        # epilogue: out = s0*(pa+bA) + s1*(pb+bB).  u/t1 run
                        # during pb's accumulation (pa stopped 16 MMs ago);
                        # only v -> o trail the last MM.  t1 on ACT (per-
                        # partition scale AP) overlaps DVE's v add.
                        u = ep.tile([128, NB], F32, tag="u")
                        nc.vector.tensor_add(u[:], pa[:], brep_sb[:, 0, nb_sl])
                        t1 = ep.tile([128, NB], F32, tag="t1")
                        nc.scalar.activation(
                            t1[:], u[:], mybir.ActivationFunctionType.Copy,
                            scale=s0,
                        )
                        # the very last group's v->o->DMA chain is the kernel
                        # tail: split it into column halves on both queues.
                        nspl = 2 if (nb == NBLK - 1 and mg == MG - 1) else 1
                        hb = NB // nspl
                        v = ep.tile([128, NB], F32, tag="v")
                        o = op.tile([128, NB], O_DT, tag="o")
                        for c in range(nspl):
                            c_sl = bass.ts(c, hb)
                            oc_sl = slice(nb * NB + c * hb, nb * NB + (c + 1) * hb)
                            nc.vector.tensor_add(
                                v[:, c_sl], pb[:, c_sl],
                                brep_sb[:, 1, oc_sl],
                            )
                            nc.vector.scalar_tensor_tensor(
                                o[:, c_sl], v[:, c_sl], s1, t1[:, c_sl],
                                op0=MULT, op1=ADD,
                            )
                            hw_eng[(mg + c) % 2].dma_start(
                                out[bass.ts(mg, 128), oc_sl], o[:, c_sl]
                            )
                w_cur = w_next

    nc.compile()
    return nc


def _host_gating(x, W_gate, b_gate):
    logits = x @ W_gate + b_gate                       # [N, 8] fp32
    m = logits.max(axis=1, keepdims=True)
    e = np.exp(logits - m)
    gates = e / e.sum(axis=1, keepdims=True)
    idx0 = np.argsort(-gates[0], kind="stable")[:2]    # token-0 top-2 experts
    scores = -np.sort(-gates, axis=1)[:, :2]           # per-token top-2 values
    return idx0, np.ascontiguousarray(scores)


def kernel(x, W_experts, b_experts, W_gate, b_gate):
    global LAST_RESULT
    x = np.ascontiguousarray(np.asarray(x, dtype=np.float32))
    W_experts = np.asarray(W_experts, dtype=np.float32)
    b_experts = np.asarray(b_experts, dtype=np.float32)
    W_gate = np.asarray(W_gate, dtype=np.float32)
    b_gate = np.asarray(b_gate, dtype=np.float32)

    idx0, scores = _host_gating(x, W_gate, b_gate)
    w_np_dt = mybir.dt.np(W_DT)
    x_np_dt = mybir.dt.np(X_DT)
    # wP[nb, e, j, p, kk, n] = W_sel[e][(j*KQ+kk)*KP + p, nb*NB + n]
    w_sel = W_experts[idx0].astype(w_np_dt)            # [2, D_IN, D_HID]
    wP = np.ascontiguousarray(
        w_sel.reshape(2, 4, KQ, KP, NBLK, NB)          # e,j,kk,p,nb,n
        .transpose(4, 0, 1, 3, 2, 5)                   # nb,e,j,p,kk,n
    )
    b2 = np.ascontiguousarray(
        b_experts[idx0][None]                          # [1, 2, D_HID]
    ).astype(np.float16)

    xT_full = x.astype(x_np_dt).T                      # [D_IN, N]

    nc = _build()
    in_maps = []
    for c in range(N_CORES):
        sl = slice(c * NT, (c + 1) * NT)
        # xQ[q, h, p, kk, t] = x[c*NT + q*TQ + t, (h*KH+kk)*KP + p]
        xQ = np.ascontiguousarray(
            xT_full[:, sl].reshape(2, KH, KP, NQ, TQ).transpose(3, 0, 2, 1, 4)
        )
        in_maps.append(
            {
                "xQ": xQ,
                "wP": wP,
                "b2": b2,
                "sC": np.ascontiguousarray(
                    scores[sl].reshape(MG, 128, 2).transpose(1, 0, 2)
                ),
            }
        )

    res = run_bass_kernel_spmd(nc, in_maps, list(range(N_CORES)))
    LAST_RESULT = res
    return np.concatenate(
        [r["out"] for r in res.results], axis=0
    ).astype(np.float32)


# revision 19
# speedup vs baseline: 1.0051x; 1.0051x over previous
"""MoE layer (top-2 of 8 experts, selection shared across tokens) on 8 TRN2 cores.

Math (faithful to the reference):
    gates = softmax(x @ W_gate + b_gate)          [N, 8]
    idx0  = top-2 expert indices of token 0       [2]
    s     = per-token top-2 gate VALUES (desc)    [N, 2]
    out   = s0 * (x @ W[A] + b[A]) + s1 * (x @ W[B] + b[B])

Strategy: gating + top-2 is 0.2% of the FLOPs -> computed on host.  The two
active expert matmuls (275 GFLOP) are data-parallel sharded over tokens across
8 cores; expert weights are replicated.  Matmuls run in fp16 (values are small,
so fp16 range is safe and its 10-bit mantissa keeps rel-err ~3e-4),
accumulating fp32 in PSUM.

The PE stream (2048 MMs x 512 cols @ 2.4 GHz = 437 us) is the hard floor;
everything else is arranged to keep the PE saturated from ~11 us on:
  - x is resident in SBUF (loaded once, 8.4 MB fp16), W streams once per
    nb-block (no x re-streaming; total DMA-in 25 MB vs 50 MB).
  - DMA is coarse (0.5 MB tiles, >=4 KB per-partition descriptor runs):
    dma_start issue costs ~0.65 us, and SDMA round-robins between queues at
    PACKET granularity where a packet is one descriptor run -- small
    descriptors starve their queue.  x/W are host-PRE-PACKED per tile.
  - The cold fill is the binding constraint for the stream start: group 0
    needs xq0 + the full 4 MB W block, so those 5 MB are split evenly
    across both HWDGE queues in exactly the order the PE consumes them.
  - bias is replicated across partitions ON-CHIP (gpsimd partition_broadcast
    from an 8 KB row) so no constant competes with the fill; per-token
    scores (sC, 16 KB) trail the critical tiles.
  - outputs are written fp16 (host upcasts) on the HWDGE queues (the SWDGE
    end-of-kernel drain costs ~6.5 us; HWDGE drains instantly).
"""

import functools

import numpy as np

import concourse.bass as bass
import concourse.mybir as mybir
import concourse.tile as tile
from concourse import bacc
from concourse.bass_utils import run_bass_kernel_spmd

N_CORES = 8
N, D_IN, D_HID = 16384, 2048, 2048
NT = N // N_CORES            # tokens per core
KP = 128                     # contraction chunk = partition dim
KCH = D_IN // KP             # 16 K-chunks
KH = 8                       # K-chunks per x half-tile
KQ = 4                       # K-chunks per W quarter-tile
NB = 512                     # output column block (1 PSUM bank of fp32)
NBLK = D_HID // NB           # 4 output blocks
TQ = 256                     # tokens per resident x tile
NQ = NT // TQ                # 8 x tiles
MPQ = TQ // 128              # m-tiles per x tile
MG = NT // 128               # 16 token groups per core

F32 = mybir.dt.float32
FP16 = mybir.dt.float16

W_DT = FP16
X_DT = FP16
O_DT = FP16

# Filled by test harness inspection: last BassKernelResults from a run.
LAST_RESULT = None


@functools.lru_cache(maxsize=1)
def _build():
    nc = bacc.Bacc("TRN2", target_bir_lowering=False, debug=False)
    # Host-pre-packed: xQ[q, h] -> one x half-tile [128, KH, TQ] (4 KB/part);
    # wP[nb, e, j] -> one W quarter-tile [128, KQ, NB] (4 KB/part).
    xQ = nc.dram_tensor("xQ", [NQ, 2, 128, KH, TQ], X_DT, kind="ExternalInput")
    wP = nc.dram_tensor("wP", [NBLK, 2, 4, 128, KQ, NB], W_DT,
                        kind="ExternalInput")
    # bias row (replicated on-chip): b2[0, e, o] = b_sel[e, o]
    b2 = nc.dram_tensor("b2", [1, 2, D_HID], FP16, kind="ExternalInput")
    # per-token scores pre-arranged on host, partition-major:
    # sC[p, m, s] = top2_score[m*128 + p, s]
    sC = nc.dram_tensor("sC", [128, MG, 2], F32, kind="ExternalInput")
    out = nc.dram_tensor("out", [NT, D_HID], O_DT, kind="ExternalOutput")

    MULT = mybir.AluOpType.mult
    ADD = mybir.AluOpType.add

    with tile.TileContext(nc) as tc:
        with (
            tc.tile_pool(name="cst", bufs=1) as cst,
            tc.tile_pool(name="xp", bufs=1) as xp,
            tc.tile_pool(name="wp", bufs=2) as wp,
            tc.tile_pool(name="ep", bufs=2) as ep,
            tc.tile_pool(name="op", bufs=3) as op,
            tc.tile_pool(name="ps", bufs=3, space=bass.MemorySpace.PSUM) as ps,
            tc.tile_pool(name="wm", bufs=1, space=bass.MemorySpace.PSUM) as wm,
        ):
            # PE warm-up: the HAM clock gate keeps the PE at 1.2 GHz until it
            # sees ~3.4 us of sustained activity, and the real stream only
            # starts once the first tiles land (~12 us) -- so the whole cold
            # fill used to run at half rate (throttle_active ~8.6 us).  Dummy
            # matmuls on (uninitialized) SBUF garbage depend on no DMA: they
            # run right out of the preamble, flip HAM to 2.4 GHz by ~11 us,
            # and their PSUM target is never read.
            warm_x = cst.tile([128, 128], X_DT)
            warm_w = cst.tile([128, NB], W_DT)
            nc.vector.memzero(warm_x[:])
            nc.vector.memzero(warm_w[:])
            warm_p = wm.tile([128, NB], F32, tag="warm")
            for _ in range(20):
                nc.tensor.matmul(
                    warm_p[:], warm_x[:], warm_w[:], start=True, stop=True
                )
            # bias: 8 KB row on the (otherwise idle) SWDGE queue, replicated
            # to all partitions by gpsimd via POOL ports -- zero SDMA traffic.
            b2_sb = cst.tile([1, 2, D_HID], FP16)
            nc.gpsimd.dma_start(b2_sb[:], b2[:])
            brep_sb = cst.tile([128, 2, D_HID], FP16)
            nc.gpsimd.partition_broadcast(brep_sb[:], b2_sb[:])

            hw_eng = [nc.sync, nc.scalar]

            def load_x(q, h, eng):
                t = xp.tile([KP, KH, TQ], X_DT, tag=f"xq{q}h{h}")
                eng.dma_start(t[:], xQ[q, h])
                return t

            def load_w(nb, e, j, eng):
                t = wp.tile([KP, KQ, NB], W_DT, tag=f"w{e}q{j}")
                eng.dma_start(t[:], wP[nb, e, j])
                return t

            # Cold fill: the 5 MB that group 0 consumes, split evenly across
            # the two queues in consumption order (pa's wa quarters on q10,
            # xq0 halves + pb's wb quarters on q1; last wb quarter evens the
            # byte split).  sC (16 KB) trails; first epilogue needs it ~30 us.
            x_t = [[None, None] for _ in range(NQ)]
            w_cur = {}
            x_t[0][0] = load_x(0, 0, nc.sync)
            w_cur[0, 0] = load_w(0, 0, 0, nc.scalar)
            x_t[0][1] = load_x(0, 1, nc.sync)
            w_cur[0, 1] = load_w(0, 0, 1, nc.scalar)
            w_cur[1, 0] = load_w(0, 1, 0, nc.sync)
            w_cur[0, 2] = load_w(0, 0, 2, nc.scalar)
            w_cur[1, 1] = load_w(0, 1, 1, nc.sync)
            w_cur[0, 3] = load_w(0, 0, 3, nc.scalar)
            w_cur[1, 2] = load_w(0, 1, 2, nc.sync)
            w_cur[1, 3] = load_w(0, 1, 3, nc.scalar)
            sC_sb = cst.tile([128, MG, 2], F32)
            nc.sync.dma_start(sC_sb[:], sC[:])
            for q in range(1, NQ):
                x_t[q][0] = load_x(q, 0, hw_eng[q % 2])
                x_t[q][1] = load_x(q, 1, hw_eng[(q + 1) % 2])

            for nb in range(NBLK):
                nb_sl = bass.ts(nb, NB)
                w_next = {} if nb + 1 < NBLK else None
                for q in range(NQ):
                    # W(nb+1) prefetch: one 0.5 MB quarter per q slice.
                    if w_next is not None:
                        e, j = q // 4, q % 4
                        w_next[e, j] = load_w(nb + 1, e, j, hw_eng[q % 2])
                    for mi in range(MPQ):
                        mg = q * MPQ + mi
                        m_sl = bass.ts(mi, 128)
                        pa = ps.tile([128, NB], F32, tag="pa")
                        pb = ps.tile([128, NB], F32, tag="pb")
                        pp = (pa, pb)
                        # expert-sequential, quarter-major: matches the cold
                        # fill's arrival order, and pa finishing early lets
                        # its epilogue ops overlap pb's accumulation.
                        for e in range(2):
                            for j in range(4):
                                for kk in range(KQ):
                                    nc.tensor.matmul(
                                        pp[e][:],
                                        x_t[q][j // 2][:, (j % 2) * KQ + kk, m_sl],
                                        w_cur[e, j][:, kk, :],
                                        start=(j == 0 and kk == 0),
                                        stop=(j == 3 and kk == KQ - 1),
                                    )
                        s0 = sC_sb[:, mg, 0:1]
                        s1 = sC_sb[:, mg, 1:2]
                        # epilogue: out = s0*(pa+bA) + s1*(pb+bB).  u/t1 run
                        # during pb's accumulation (pa stopped 16 MMs ago);
                        # only v -> o trail the last MM.  t1 on ACT (per-
                        # partition scale AP) overlaps DVE's v add.
                        u = ep.tile([128, NB], F32, tag="u")
                        nc.vector.tensor_add(u[:], pa[:], brep_sb[:, 0, nb_sl])
                        t1 = ep.tile([128, NB], F32, tag="t1")
                        nc.scalar.activation(
                            t1[:], u[:], mybir.ActivationFunctionType.Copy,
                            scale=s0,
                        )
                        # the very last group's v->o->DMA chain is the kernel
                        # tail: split it into column halves on both queues.
                        nspl = 2 if (nb == NBLK - 1 and mg == MG - 1) else 1
                        hb = NB // nspl
                        v = ep.tile([128, NB], F32, tag="v")
                        o = op.tile([128, NB], O_DT, tag="o")
                        for c in range(nspl):
                            c_sl = bass.ts(c, hb)
                            oc_sl = slice(nb * NB + c * hb, nb * NB + (c + 1) * hb)
                            nc.vector.tensor_add(
                                v[:, c_sl], pb[:, c_sl],
                                brep_sb[:, 1, oc_sl],
                            )
                            nc.vector.scalar_tensor_tensor(
                                o[:, c_sl], v[:, c_sl], s1, t1[:, c_sl],
                                op0=MULT, op1=ADD,
                            )
                            hw_eng[(mg + c) % 2].dma_start(
                                out[bass.ts(mg, 128), oc_sl], o[:, c_sl]
                            )
                w_cur = w_next

    nc.compile()
    return nc


def _host_gating(x, W_gate, b_gate):
    logits = x @ W_gate + b_gate                       # [N, 8] fp32
    m = logits.max(axis=1, keepdims=True)
    e = np.exp(logits - m)
    gates = e / e.sum(axis=1, keepdims=True)
    idx0 = np.argsort(-gates[0], kind="stable")[:2]    # token-0 top-2 experts
    scores = -np.sort(-gates, axis=1)[:, :2]           # per-token top-2 values
    return idx0, np.ascontiguousarray(scores)


def kernel(x, W_experts, b_experts, W_gate, b_gate):
    global LAST_RESULT
    x = np.ascontiguousarray(np.asarray(x, dtype=np.float32))
    W_experts = np.asarray(W_experts, dtype=np.float32)
    b_experts = np.asarray(b_experts, dtype=np.float32)
    W_gate = np.asarray(W_gate, dtype=np.float32)
    b_gate = np.asarray(b_gate, dtype=np.float32)

    idx0, scores = _host_gating(x, W_gate, b_gate)
    w_np_dt = mybir.dt.np(W_DT)
    x_np_dt = mybir.dt.np(X_DT)
    # wP[nb, e, j, p, kk, n] = W_sel[e][(j*KQ+kk)*KP + p, nb*NB + n]
    w_sel = W_experts[idx0].astype(w_np_dt)            # [2, D_IN, D_HID]
    wP = np.ascontiguousarray(
        w_sel.reshape(2, 4, KQ, KP, NBLK, NB)          # e,j,kk,p,nb,n
        .transpose(4, 0, 1, 3, 2, 5)                   # nb,e,j,p,kk,n
    )
    b2 = np.ascontiguousarray(
        b_experts[idx0][None]                          # [1, 2, D_HID]
    ).astype(np.float16)

    xT_full = x.astype(x_np_dt).T                      # [D_IN, N]

    nc = _build()
    in_maps = []
    for c in range(N_CORES):
        sl = slice(c * NT, (c + 1) * NT)
        # xQ[q, h, p, kk, t] = x[c*NT + q*TQ + t, (h*KH+kk)*KP + p]
        xQ = np.ascontiguousarray(
            xT_full[:, sl].reshape(2, KH, KP, NQ, TQ).transpose(3, 0, 2, 1, 4)
        )
        in_maps.append(
            {
                "xQ": xQ,
                "wP": wP,
                "b2": b2,
                "sC": np.ascontiguousarray(
                    scores[sl].reshape(MG, 128, 2).transpose(1, 0, 2)
                ),
            }
        )

    res = run_bass_kernel_spmd(nc, in_maps, list(range(N_CORES)))
    LAST_RESULT = res
    return np.concatenate(
        [r["out"] for r in res.results], axis=0
    ).astype(np.float32)


# revision 23
# speedup vs baseline: 1.0071x; 1.0021x over previous
"""MoE layer (top-2 of 8 experts, selection shared across tokens) on 8 TRN2 cores.

Math (faithful to the reference):
    gates = softmax(x @ W_gate + b_gate)          [N, 8]
    idx0  = top-2 expert indices of token 0       [2]
    s     = per-token top-2 gate VALUES (desc)    [N, 2]
    out   = s0 * (x @ W[A] + b[A]) + s1 * (x @ W[B] + b[B])

Strategy: gating + top-2 is 0.2% of the FLOPs -> computed on host.  The two
active expert matmuls (275 GFLOP) are data-parallel sharded over tokens across
8 cores; expert weights are replicated.  Matmuls run in fp16 (values are small,
so fp16 range is safe and its 10-bit mantissa keeps rel-err ~3e-4),
accumulating fp32 in PSUM.

The PE stream (2048 MMs x 512 cols @ 2.4 GHz = 437 us) is the hard floor;
everything else is arranged to keep the PE saturated from ~11 us on:
  - x is resident in SBUF (loaded once, 8.4 MB fp16), W streams once per
    nb-block (no x re-streaming; total DMA-in 25 MB vs 50 MB).
  - DMA is coarse (0.5 MB tiles, >=4 KB per-partition descriptor runs):
    dma_start issue costs ~0.65 us, and SDMA round-robins between queues at
    PACKET granularity where a packet is one descriptor run -- small
    descriptors starve their queue.  x/W are host-PRE-PACKED per tile.
  - The cold fill is the binding constraint for the stream start: group 0
    needs xq0 + the full 4 MB W block, so those 5 MB are split evenly
    across both HWDGE queues in exactly the order the PE consumes them.
  - bias is replicated across partitions ON-CHIP (gpsimd partition_broadcast
    from an 8 KB row) so no constant competes with the fill; per-token
    scores (sC, 16 KB) trail the critical tiles.
  - outputs are written fp16 (host upcasts) on the HWDGE queues (the SWDGE
    end-of-kernel drain costs ~6.5 us; HWDGE drains instantly).
"""

import functools

import numpy as np

import concourse.bass as bass
import concourse.mybir as mybir
import concourse.tile as tile
from concourse import bacc
from concourse.bass_utils import run_bass_kernel_spmd

N_CORES = 8
N, D_IN, D_HID = 16384, 2048, 2048
NT = N // N_CORES            # tokens per core
KP = 128                     # contraction chunk = partition dim
KCH = D_IN // KP             # 16 K-chunks
KH = 8                       # K-chunks per x half-tile
KQ = 4                       # K-chunks per W quarter-tile
NB = 512                     # output column block (1 PSUM bank of fp32)
NBLK = D_HID // NB           # 4 output blocks
TQ = 256                     # tokens per resident x tile
NQ = NT // TQ                # 8 x tiles
MPQ = TQ // 128              # m-tiles per x tile
MG = NT // 128               # 16 token groups per core

F32 = mybir.dt.float32
FP16 = mybir.dt.float16

W_DT = FP16
X_DT = FP16
O_DT = FP16

# Filled by test harness inspection: last BassKernelResults from a run.
LAST_RESULT = None


@functools.lru_cache(maxsize=1)
def _build():
    nc = bacc.Bacc("TRN2", target_bir_lowering=False, debug=False)
    # Host-pre-packed: xQ[q, h] -> one x half-tile [128, KH, TQ] (4 KB/part);
    # wP[nb, e, j] -> one W quarter-tile [128, KQ, NB] (4 KB/part).
    xQ = nc.dram_tensor("xQ", [NQ, 2, 128, KH, TQ], X_DT, kind="ExternalInput")
    wP = nc.dram_tensor("wP", [NBLK, 2, 4, 128, KQ, NB], W_DT,
                        kind="ExternalInput")
    # bias row (replicated on-chip): b2[0, e, o] = b_sel[e, o]
    b2 = nc.dram_tensor("b2", [1, 2, D_HID], FP16, kind="ExternalInput")
    # per-token scores pre-arranged on host, partition-major:
    # sC[p, m, s] = top2_score[m*128 + p, s]
    sC = nc.dram_tensor("sC", [128, MG, 2], F32, kind="ExternalInput")
    out = nc.dram_tensor("out", [NT, D_HID], O_DT, kind="ExternalOutput")

    MULT = mybir.AluOpType.mult
    ADD = mybir.AluOpType.add

    with tile.TileContext(nc) as tc:
        with (
            tc.tile_pool(name="cst", bufs=1) as cst,
            tc.tile_pool(name="xp", bufs=1) as xp,
            tc.tile_pool(name="wp", bufs=2) as wp,
            tc.tile_pool(name="ep", bufs=2) as ep,
            tc.tile_pool(name="op", bufs=3) as op,
            tc.tile_pool(name="ps", bufs=2, space=bass.MemorySpace.PSUM) as ps,
        ):
            # PE warm-up: the HAM clock gate keeps the PE at 1.2 GHz until it
            # sees ~3.4 us of sustained activity, and the first real tiles
            # only land at ~10 us -- without this the whole cold fill ran at
            # half rate.  Dummy matmuls on zeroed SBUF depend on no DMA: they
            # run right out of the preamble (~7.5 us) and flip HAM to 2.4 GHz
            # by ~11 us.  Their PSUM target is slot 0 of the p00 ring, whose
            # first real user (q=1) starts long after they retire.
            warm_x = cst.tile([128, 128], X_DT)
            warm_w = cst.tile([128, NB], W_DT)
            nc.vector.memzero(warm_x[:])
            nc.vector.memzero(warm_w[:])
            warm_p = ps.tile([128, NB], F32, tag="p00")
            for _ in range(12):
                nc.tensor.matmul(
                    warm_p[:], warm_x[:], warm_w[:], start=True, stop=True
                )
            # bias: 8 KB row on the (otherwise idle) SWDGE queue, replicated
            # to all partitions by gpsimd via POOL ports -- zero SDMA traffic.
            b2_sb = cst.tile([1, 2, D_HID], FP16)
            nc.gpsimd.dma_start(b2_sb[:], b2[:])
            brep_sb = cst.tile([128, 2, D_HID], FP16)
            nc.gpsimd.partition_broadcast(brep_sb[:], b2_sb[:])

            hw_eng = [nc.sync, nc.scalar]

            def load_x(q, h, eng):
                t = xp.tile([KP, KH, TQ], X_DT, tag=f"xq{q}h{h}")
                eng.dma_start(t[:], xQ[q, h])
                return t

            def load_w(nb, e, j, eng):
                t = wp.tile([KP, KQ, NB], W_DT, tag=f"w{e}q{j}")
                eng.dma_start(t[:], wP[nb, e, j])
                return t

            # Cold fill: the 5 MB that the q=0 groups consume, split evenly
            # across the two queues in exactly the order the (mi-interleaved,
            # expert-major) MM loop consumes it: xh0+waj0 first, then wa
            # quarters alternating with xh1, then wb quarters.  sC (16 KB)
            # trails; the first epilogue needs it at ~26 us.
            x_t = [[None, None] for _ in range(NQ)]
            w_cur = {}
            x_t[0][0] = load_x(0, 0, nc.sync)
            w_cur[0, 0] = load_w(0, 0, 0, nc.scalar)
            w_cur[0, 1] = load_w(0, 0, 1, nc.sync)
            x_t[0][1] = load_x(0, 1, nc.scalar)
            w_cur[0, 3] = load_w(0, 0, 3, nc.sync)
            w_cur[0, 2] = load_w(0, 0, 2, nc.scalar)
            w_cur[1, 1] = load_w(0, 1, 1, nc.sync)
            w_cur[1, 0] = load_w(0, 1, 0, nc.scalar)
            w_cur[1, 3] = load_w(0, 1, 3, nc.sync)
            w_cur[1, 2] = load_w(0, 1, 2, nc.scalar)
            sC_sb = cst.tile([128, MG, 2], F32)
            nc.sync.dma_start(sC_sb[:], sC[:])
            for q in range(1, NQ):
                x_t[q][0] = load_x(q, 0, hw_eng[q % 2])
                x_t[q][1] = load_x(q, 1, hw_eng[(q + 1) % 2])

            for nb in range(NBLK):
                nb_sl = bass.ts(nb, NB)
                w_next = {} if nb + 1 < NBLK else None
                for q in range(NQ):
                    # W(nb+1) prefetch: one 0.5 MB quarter per q slice.
                    if w_next is not None:
                        e, j = q // 4, q % 4
                        w_next[e, j] = load_w(nb + 1, e, j, hw_eng[q % 2])
                    # Both mi-groups interleaved per W quarter: the PE then
                    # consumes each arriving 0.5 MB quarter with 8 MMs
                    # (1.73 us >= the ~1.35 us arrival cadence), so the cold
                    # fill runs gap-free and HAM never re-throttles.  Expert-
                    # major: pa* stop 16 MMs early, so their epilogue ops
                    # overlap pb*'s accumulation.
                    pt = {
                        (e, mi): ps.tile(
                            [128, NB], F32, tag=f"p{e}{mi}", name=f"pt{e}{mi}"
                        )
                        for e in range(2)
                        for mi in range(MPQ)
                    }
                    for e in range(2):
                        for j in range(4):
                            for mi in range(MPQ):
                                for kk in range(KQ):
                                    nc.tensor.matmul(
                                        pt[e, mi][:],
                                        x_t[q][j // 2][
                                            :, (j % 2) * KQ + kk, bass.ts(mi, 128)
                                        ],
                                        w_cur[e, j][:, kk, :],
                                        start=(j == 0 and kk == 0),
                                        stop=(j == 3 and kk == KQ - 1),
                                    )
                    for mi in range(MPQ):
                        mg = q * MPQ + mi
                        pa, pb = pt[0, mi], pt[1, mi]
                        s0 = sC_sb[:, mg, 0:1]
                        s1 = sC_sb[:, mg, 1:2]
                        # epilogue: out = s0*(pa+bA) + s1*(pb+bB).  u/t1 run
                        # during pb's accumulation (pa stopped 16 MMs ago);
                        # only v -> o trail the last MM.  t1 on ACT (per-
                        # partition scale AP) overlaps DVE's v add.
                        u = ep.tile([128, NB], F32, tag="u")
                        nc.vector.tensor_add(u[:], pa[:], brep_sb[:, 0, nb_sl])
                        t1 = ep.tile([128, NB], F32, tag="t1")
                        nc.scalar.activation(
                            t1[:], u[:], mybir.ActivationFunctionType.Copy,
                            scale=s0,
                        )
                        # the very last group's v->o->DMA chain is the kernel
                        # tail: split it into column halves on both queues.
                        nspl = 2 if (nb == NBLK - 1 and mg == MG - 1) else 1
                        hb = NB // nspl
                        v = ep.tile([128, NB], F32, tag="v")
                        o = op.tile([128, NB], O_DT, tag="o")
                        for c in range(nspl):
                            c_sl = bass.ts(c, hb)
                            oc_sl = slice(nb * NB + c * hb, nb * NB + (c + 1) * hb)
                            nc.vector.tensor_add(
                                v[:, c_sl], pb[:, c_sl],
                                brep_sb[:, 1, oc_sl],
                            )
                            nc.vector.scalar_tensor_tensor(
                                o[:, c_sl], v[:, c_sl], s1, t1[:, c_sl],
                                op0=MULT, op1=ADD,
                            )
                            hw_eng[(mg + c) % 2].dma_start(
                                out[bass.ts(mg, 128), oc_sl], o[:, c_sl]
                            )
                w_cur = w_next

    nc.compile()
    return nc


def _host_gating(x, W_gate, b_gate):
    logits = x @ W_gate + b_gate                       # [N, 8] fp32
    m = logits.max(axis=1, keepdims=True)
    e = np.exp(logits - m)
    gates = e / e.sum(axis=1, keepdims=True)
    idx0 = np.argsort(-gates[0], kind="stable")[:2]    # token-0 top-2 experts
    scores = -np.sort(-gates, axis=1)[:, :2]           # per-token top-2 values
    return idx0, np.ascontiguousarray(scores)


def kernel(x, W_experts, b_experts, W_gate, b_gate):
    global LAST_RESULT
    x = np.ascontiguousarray(np.asarray(x, dtype=np.float32))
    W_experts = np.asarray(W_experts, dtype=np.float32)
    b_experts = np.asarray(b_experts, dtype=np.float32)
    W_gate = np.asarray(W_gate, dtype=np.float32)
    b_gate = np.asarray(b_gate, dtype=np.float32)

    idx0, scores = _host_gating(x, W_gate, b_gate)
    w_np_dt = mybir.dt.np(W_DT)
    x_np_dt = mybir.dt.np(X_DT)
    # wP[nb, e, j, p, kk, n] = W_sel[e][(j*KQ+kk)*KP + p, nb*NB + n]
    w_sel = W_experts[idx0].astype(w_np_dt)            # [2, D_IN, D_HID]
    wP = np.ascontiguousarray(
        w_sel.reshape(2, 4, KQ, KP, NBLK, NB)          # e,j,kk,p,nb,n
        .transpose(4, 0, 1, 3, 2, 5)                   # nb,e,j,p,kk,n
    )
    b2 = np.ascontiguousarray(
        b_experts[idx0][None]                          # [1, 2, D_HID]
    ).astype(np.float16)

    xT_full = x.astype(x_np_dt).T                      # [D_IN, N]

    nc = _build()
    in_maps = []
    for c in range(N_CORES):
        sl = slice(c * NT, (c + 1) * NT)
        # xQ[q, h, p, kk, t] = x[c*NT + q*TQ + t, (h*KH+kk)*KP + p]
        xQ = np.ascontiguousarray(
            xT_full[:, sl].reshape(2, KH, KP, NQ, TQ).transpose(3, 0, 2, 1, 4)
        )
        in_maps.append(
            {
                "xQ": xQ,
                "wP": wP,
                "b2": b2,
                "sC": np.ascontiguousarray(
                    scores[sl].reshape(MG, 128, 2).transpose(1, 0, 2)
                ),
            }
        )

    res = run_bass_kernel_spmd(nc, in_maps, list(range(N_CORES)))
    LAST_RESULT = res
    return np.concatenate(
        [r["out"] for r in res.results], axis=0
    ).astype(np.float32)


# revision 24
# speedup vs baseline: 1.0083x; 1.0012x over previous
"""MoE layer (top-2 of 8 experts, selection shared across tokens) on 8 TRN2 cores.

Math (faithful to the reference):
    gates = softmax(x @ W_gate + b_gate)          [N, 8]
    idx0  = top-2 expert indices of token 0       [2]
    s     = per-token top-2 gate VALUES (desc)    [N, 2]
    out   = s0 * (x @ W[A] + b[A]) + s1 * (x @ W[B] + b[B])

Strategy: gating + top-2 is 0.2% of the FLOPs -> computed on host.  The two
active expert matmuls (275 GFLOP) are data-parallel sharded over tokens across
8 cores; expert weights are replicated.  Matmuls run in fp16 (values are small,
so fp16 range is safe and its 10-bit mantissa keeps rel-err ~3e-4),
accumulating fp32 in PSUM.

The PE stream (2048 MMs x 512 cols @ 2.4 GHz = 437 us) is the hard floor;
everything else is arranged to keep the PE saturated from ~11 us on:
  - x is resident in SBUF (loaded once, 8.4 MB fp16), W streams once per
    nb-block (no x re-streaming; total DMA-in 25 MB vs 50 MB).
  - DMA is coarse (0.5 MB tiles, >=4 KB per-partition descriptor runs):
    dma_start issue costs ~0.65 us, and SDMA round-robins between queues at
    PACKET granularity where a packet is one descriptor run -- small
    descriptors starve their queue.  x/W are host-PRE-PACKED per tile.
  - The cold fill is the binding constraint for the stream start: group 0
    needs xq0 + the full 4 MB W block, so those 5 MB are split evenly
    across both HWDGE queues in exactly the order the PE consumes them.
  - bias is replicated across partitions ON-CHIP (gpsimd partition_broadcast
    from an 8 KB row) so no constant competes with the fill; per-token
    scores (sC, 16 KB) trail the critical tiles.
  - outputs are written fp16 (host upcasts) on the HWDGE queues (the SWDGE
    end-of-kernel drain costs ~6.5 us; HWDGE drains instantly).
"""

import functools

import numpy as np

import concourse.bass as bass
import concourse.mybir as mybir
import concourse.tile as tile
from concourse import bacc
from concourse.bass_utils import run_bass_kernel_spmd

N_CORES = 8
N, D_IN, D_HID = 16384, 2048, 2048
NT = N // N_CORES            # tokens per core
KP = 128                     # contraction chunk = partition dim
KCH = D_IN // KP             # 16 K-chunks
KH = 8                       # K-chunks per x half-tile
KQ = 4                       # K-chunks per W quarter-tile
NB = 512                     # output column block (1 PSUM bank of fp32)
NBLK = D_HID // NB           # 4 output blocks
TQ = 256                     # tokens per resident x tile
NQ = NT // TQ                # 8 x tiles
MPQ = TQ // 128              # m-tiles per x tile
MG = NT // 128               # 16 token groups per core

F32 = mybir.dt.float32
FP16 = mybir.dt.float16

W_DT = FP16
X_DT = FP16
O_DT = FP16

# Filled by test harness inspection: last BassKernelResults from a run.
LAST_RESULT = None


@functools.lru_cache(maxsize=1)
def _build():
    nc = bacc.Bacc("TRN2", target_bir_lowering=False, debug=False)
    # Host-pre-packed: xQ[q, h] -> one x half-tile [128, KH, TQ] (4 KB/part);
    # wP[nb, e, j] -> one W quarter-tile [128, KQ, NB] (4 KB/part).
    xQ = nc.dram_tensor("xQ", [NQ, 2, 128, KH, TQ], X_DT, kind="ExternalInput")
    wP = nc.dram_tensor("wP", [NBLK, 2, 4, 128, KQ, NB], W_DT,
                        kind="ExternalInput")
    # bias row (replicated on-chip): b2[0, e, o] = b_sel[e, o]
    b2 = nc.dram_tensor("b2", [1, 2, D_HID], FP16, kind="ExternalInput")
    # per-token scores pre-arranged on host, partition-major:
    # sC[p, m, s] = top2_score[m*128 + p, s]
    sC = nc.dram_tensor("sC", [128, MG, 2], F32, kind="ExternalInput")
    out = nc.dram_tensor("out", [NT, D_HID], O_DT, kind="ExternalOutput")

    MULT = mybir.AluOpType.mult
    ADD = mybir.AluOpType.add

    with tile.TileContext(nc) as tc:
        with (
            tc.tile_pool(name="cst", bufs=1) as cst,
            tc.tile_pool(name="xp", bufs=1) as xp,
            tc.tile_pool(name="wp", bufs=2) as wp,
            tc.tile_pool(name="ep", bufs=2) as ep,
            tc.tile_pool(name="op", bufs=3) as op,
            tc.tile_pool(name="ps", bufs=2, space=bass.MemorySpace.PSUM) as ps,
        ):
            # PE warm-up: the HAM clock gate keeps the PE at 1.2 GHz until it
            # sees ~3.4 us of sustained activity, and the first real tiles
            # only land at ~10 us -- without this the whole cold fill ran at
            # half rate.  Dummy matmuls on zeroed SBUF depend on no DMA: they
            # run right out of the preamble (~7.5 us) and flip HAM to 2.4 GHz
            # by ~11 us.  Their PSUM target is slot 0 of the p00 ring, whose
            # first real user (q=1) starts long after they retire.
            warm_x = cst.tile([128, 128], X_DT)
            warm_w = cst.tile([128, NB], W_DT)
            nc.vector.memzero(warm_x[:])
            nc.vector.memzero(warm_w[:])
            warm_p = ps.tile([128, NB], F32, tag="p00")
            for _ in range(12):
                nc.tensor.matmul(
                    warm_p[:], warm_x[:], warm_w[:], start=True, stop=True
                )
            # bias: 8 KB row on the (otherwise idle) SWDGE queue, replicated
            # to all partitions by gpsimd via POOL ports -- zero SDMA traffic.
            b2_sb = cst.tile([1, 2, D_HID], FP16)
            nc.gpsimd.dma_start(b2_sb[:], b2[:])
            brep_sb = cst.tile([128, 2, D_HID], FP16)
            nc.gpsimd.partition_broadcast(brep_sb[:], b2_sb[:])

            hw_eng = [nc.sync, nc.scalar]

            def load_x(q, h, eng):
                t = xp.tile([KP, KH, TQ], X_DT, tag=f"xq{q}h{h}")
                eng.dma_start(t[:], xQ[q, h])
                return t

            def load_w(nb, e, j, eng):
                t = wp.tile([KP, KQ, NB], W_DT, tag=f"w{e}q{j}")
                eng.dma_start(t[:], wP[nb, e, j])
                return t

            # Cold fill: the 5 MB that the q=0 groups consume, split evenly
            # across the two queues in exactly the order the (mi-interleaved,
            # expert-major) MM loop consumes it: xh0+waj0 first, then wa
            # quarters alternating with xh1, then wb quarters.  sC (16 KB)
            # trails; the first epilogue needs it at ~26 us.
            x_t = [[None, None] for _ in range(NQ)]
            w_cur = {}
            x_t[0][0] = load_x(0, 0, nc.sync)
            w_cur[0, 0] = load_w(0, 0, 0, nc.scalar)
            w_cur[0, 1] = load_w(0, 0, 1, nc.sync)
            x_t[0][1] = load_x(0, 1, nc.scalar)
            w_cur[0, 3] = load_w(0, 0, 3, nc.sync)
            w_cur[0, 2] = load_w(0, 0, 2, nc.scalar)
            w_cur[1, 1] = load_w(0, 1, 1, nc.sync)
            w_cur[1, 0] = load_w(0, 1, 0, nc.scalar)
            w_cur[1, 3] = load_w(0, 1, 3, nc.sync)
            w_cur[1, 2] = load_w(0, 1, 2, nc.scalar)
            sC_sb = cst.tile([128, MG, 2], F32)
            nc.sync.dma_start(sC_sb[:], sC[:])
            for q in range(1, NQ):
                x_t[q][0] = load_x(q, 0, hw_eng[q % 2])
                x_t[q][1] = load_x(q, 1, hw_eng[(q + 1) % 2])

            for nb in range(NBLK):
                nb_sl = bass.ts(nb, NB)
                w_next = {} if nb + 1 < NBLK else None
                for q in range(NQ):
                    # W(nb+1) prefetch: one 0.5 MB quarter per q slice.
                    if w_next is not None:
                        e, j = q // 4, q % 4
                        w_next[e, j] = load_w(nb + 1, e, j, hw_eng[q % 2])
                    # Both mi-groups interleaved per W quarter: the PE then
                    # consumes each arriving 0.5 MB quarter with 8 MMs
                    # (1.73 us >= the ~1.35 us arrival cadence), so the cold
                    # fill runs gap-free and HAM never re-throttles.  Expert-
                    # major: pa* stop 16 MMs early, so their epilogue ops
                    # overlap pb*'s accumulation.
                    pt = {
                        (e, mi): ps.tile(
                            [128, NB], F32, tag=f"p{e}{mi}", name=f"pt{e}{mi}"
                        )
                        for e in range(2)
                        for mi in range(MPQ)
                    }
                    for e in range(2):
                        for j in range(4):
                            # dummy heartbeats between the first group's
                            # quarter-blocks: when a quarter's DMA is late,
                            # these run in the gap and keep the HAM activity
                            # window busy (a single mid-fill re-throttle to
                            # 1.2 GHz costs ~6.5 us of wall time).
                            if nb == 0 and q == 0 and (e, j) != (0, 0):
                                for _ in range(2):
                                    nc.tensor.matmul(
                                        warm_p[:], warm_x[:], warm_w[:],
                                        start=True, stop=True,
                                    )
                            for mi in range(MPQ):
                                for kk in range(KQ):
                                    nc.tensor.matmul(
                                        pt[e, mi][:],
                                        x_t[q][j // 2][
                                            :, (j % 2) * KQ + kk, bass.ts(mi, 128)
                                        ],
                                        w_cur[e, j][:, kk, :],
                                        start=(j == 0 and kk == 0),
                                        stop=(j == 3 and kk == KQ - 1),
                                    )
                    for mi in range(MPQ):
                        mg = q * MPQ + mi
                        pa, pb = pt[0, mi], pt[1, mi]
                        s0 = sC_sb[:, mg, 0:1]
                        s1 = sC_sb[:, mg, 1:2]
                        # epilogue: out = s0*(pa+bA) + s1*(pb+bB).  u/t1 run
                        # during pb's accumulation (pa stopped 16 MMs ago);
                        # only v -> o trail the last MM.  t1 on ACT (per-
                        # partition scale AP) overlaps DVE's v add.
                        u = ep.tile([128, NB], F32, tag="u")
                        nc.vector.tensor_add(u[:], pa[:], brep_sb[:, 0, nb_sl])
                        t1 = ep.tile([128, NB], F32, tag="t1")
                        nc.scalar.activation(
                            t1[:], u[:], mybir.ActivationFunctionType.Copy,
                            scale=s0,
                        )
                        # the very last group's v->o->DMA chain is the kernel
                        # tail: split it into column halves on both queues.
                        nspl = 2 if (nb == NBLK - 1 and mg == MG - 1) else 1
                        hb = NB // nspl
                        v = ep.tile([128, NB], F32, tag="v")
                        o = op.tile([128, NB], O_DT, tag="o")
                        for c in range(nspl):
                            c_sl = bass.ts(c, hb)
                            oc_sl = slice(nb * NB + c * hb, nb * NB + (c + 1) * hb)
                            nc.vector.tensor_add(
                                v[:, c_sl], pb[:, c_sl],
                                brep_sb[:, 1, oc_sl],
                            )
                            nc.vector.scalar_tensor_tensor(
                                o[:, c_sl], v[:, c_sl], s1, t1[:, c_sl],
                                op0=MULT, op1=ADD,
                            )
                            hw_eng[(mg + c) % 2].dma_start(
                                out[bass.ts(mg, 128), oc_sl], o[:, c_sl]
                            )
                w_cur = w_next

    nc.compile()
    return nc


def _host_gating(x, W_gate, b_gate):
    logits = x @ W_gate + b_gate                       # [N, 8] fp32
    m = logits.max(axis=1, keepdims=True)
    e = np.exp(logits - m)
    gates = e / e.sum(axis=1, keepdims=True)
    idx0 = np.argsort(-gates[0], kind="stable")[:2]    # token-0 top-2 experts
    scores = -np.sort(-gates, axis=1)[:, :2]           # per-token top-2 values
    return idx0, np.ascontiguousarray(scores)


def kernel(x, W_experts, b_experts, W_gate, b_gate):
    global LAST_RESULT
    x = np.ascontiguousarray(np.asarray(x, dtype=np.float32))
    W_experts = np.asarray(W_experts, dtype=np.float32)
    b_experts = np.asarray(b_experts, dtype=np.float32)
    W_gate = np.asarray(W_gate, dtype=np.float32)
    b_gate = np.asarray(b_gate, dtype=np.float32)

    idx0, scores = _host_gating(x, W_gate, b_gate)
    w_np_dt = mybir.dt.np(W_DT)
    x_np_dt = mybir.dt.np(X_DT)
    # wP[nb, e, j, p, kk, n] = W_sel[e][(j*KQ+kk)*KP + p, nb*NB + n]
    w_sel = W_experts[idx0].astype(w_np_dt)            # [2, D_IN, D_HID]
    wP = np.ascontiguousarray(
        w_sel.reshape(2, 4, KQ, KP, NBLK, NB)          # e,j,kk,p,nb,n
        .transpose(4, 0, 1, 3, 2, 5)                   # nb,e,j,p,kk,n
    )
    b2 = np.ascontiguousarray(
        b_experts[idx0][None]                          # [1, 2, D_HID]
    ).astype(np.float16)

    xT_full = x.astype(x_np_dt).T                      # [D_IN, N]

    nc = _build()
    in_maps = []
    for c in range(N_CORES):
        sl = slice(c * NT, (c + 1) * NT)
        # xQ[q, h, p, kk, t] = x[c*NT + q*TQ + t, (h*KH+kk)*KP + p]
        xQ = np.ascontiguousarray(
            xT_full[:, sl].reshape(2, KH, KP, NQ, TQ).transpose(3, 0, 2, 1, 4)
        )
        in_maps.append(
            {
                "xQ": xQ,
                "wP": wP,
                "b2": b2,
                "sC": np.ascontiguousarray(
                    scores[sl].reshape(MG, 128, 2).transpose(1, 0, 2)
                ),
            }
        )

    res = run_bass_kernel_spmd(nc, in_maps, list(range(N_CORES)))
    LAST_RESULT = res
    return np.concatenate(
        [r["out"] for r in res.results], axis=0
    ).astype(np.float32)


# revision 25
# speedup vs baseline: 1.0085x; 1.0001x over previous
"""MoE layer (top-2 of 8 experts, selection shared across tokens) on 8 TRN2 cores.

Math (faithful to the reference):
    gates = softmax(x @ W_gate + b_gate)          [N, 8]
    idx0  = top-2 expert indices of token 0       [2]
    s     = per-token top-2 gate VALUES (desc)    [N, 2]
    out   = s0 * (x @ W[A] + b[A]) + s1 * (x @ W[B] + b[B])

Strategy: gating + top-2 is 0.2% of the FLOPs -> computed on host.  The two
active expert matmuls (275 GFLOP) are data-parallel sharded over tokens across
8 cores; expert weights are replicated.  Matmuls run in fp16 (values are small,
so fp16 range is safe and its 10-bit mantissa keeps rel-err ~3e-4),
accumulating fp32 in PSUM.

The PE stream (2048 MMs x 512 cols @ 2.4 GHz = 437 us) is the hard floor;
everything else is arranged to keep the PE saturated from ~11 us on:
  - x is resident in SBUF (loaded once, 8.4 MB fp16), W streams once per
    nb-block (no x re-streaming; total DMA-in 25 MB vs 50 MB).
  - DMA is coarse (0.5 MB tiles, >=4 KB per-partition descriptor runs):
    dma_start issue costs ~0.65 us, and SDMA round-robins between queues at
    PACKET granularity where a packet is one descriptor run -- small
    descriptors starve their queue.  x/W are host-PRE-PACKED per tile.
  - The cold fill is the binding constraint for the stream start: group 0
    needs xq0 + the full 4 MB W block, so those 5 MB are split evenly
    across both HWDGE queues in exactly the order the PE consumes them.
  - bias is replicated across partitions ON-CHIP (gpsimd partition_broadcast
    from an 8 KB row) so no constant competes with the fill; per-token
    scores (sC, 16 KB) trail the critical tiles.
  - outputs are written fp16 (host upcasts) on the HWDGE queues (the SWDGE
    end-of-kernel drain costs ~6.5 us; HWDGE drains instantly).
"""

import functools

import numpy as np

import concourse.bass as bass
import concourse.mybir as mybir
import concourse.tile as tile
from concourse import bacc
from concourse.bass_utils import run_bass_kernel_spmd

N_CORES = 8
N, D_IN, D_HID = 16384, 2048, 2048
NT = N // N_CORES            # tokens per core
KP = 128                     # contraction chunk = partition dim
KCH = D_IN // KP             # 16 K-chunks
KH = 8                       # K-chunks per x half-tile
KQ = 4                       # K-chunks per W quarter-tile
NB = 512                     # output column block (1 PSUM bank of fp32)
NBLK = D_HID // NB           # 4 output blocks
TQ = 256                     # tokens per resident x tile
NQ = NT // TQ                # 8 x tiles
MPQ = TQ // 128              # m-tiles per x tile
MG = NT // 128               # 16 token groups per core

F32 = mybir.dt.float32
FP16 = mybir.dt.float16

W_DT = FP16
X_DT = FP16
O_DT = FP16

# Filled by test harness inspection: last BassKernelResults from a run.
LAST_RESULT = None


@functools.lru_cache(maxsize=1)
def _build():
    nc = bacc.Bacc("TRN2", target_bir_lowering=False, debug=False)
    # Host-pre-packed: xQ[q, h] -> one x half-tile [128, KH, TQ] (4 KB/part);
    # wP[nb, e, j] -> one W quarter-tile [128, KQ, NB] (4 KB/part).
    xQ = nc.dram_tensor("xQ", [NQ, 2, 128, KH, TQ], X_DT, kind="ExternalInput")
    wP = nc.dram_tensor("wP", [NBLK, 2, 4, 128, KQ, NB], W_DT,
                        kind="ExternalInput")
    # bias row (replicated on-chip): b2[0, e, o] = b_sel[e, o]
    b2 = nc.dram_tensor("b2", [1, 2, D_HID], FP16, kind="ExternalInput")
    # per-token scores pre-arranged on host, partition-major:
    # sC[p, m, s] = top2_score[m*128 + p, s]
    sC = nc.dram_tensor("sC", [128, MG, 2], F32, kind="ExternalInput")
    out = nc.dram_tensor("out", [NT, D_HID], O_DT, kind="ExternalOutput")

    MULT = mybir.AluOpType.mult
    ADD = mybir.AluOpType.add

    with tile.TileContext(nc) as tc:
        with (
            tc.tile_pool(name="cst", bufs=1) as cst,
            tc.tile_pool(name="xp", bufs=1) as xp,
            tc.tile_pool(name="wp", bufs=2) as wp,
            tc.tile_pool(name="ep", bufs=2) as ep,
            tc.tile_pool(name="op", bufs=3) as op,
            tc.tile_pool(name="ps", bufs=2, space=bass.MemorySpace.PSUM) as ps,
        ):
            # PE warm-up: the HAM clock gate keeps the PE at 1.2 GHz until it
            # sees ~3.4 us of sustained activity, and the first real tiles
            # only land at ~10 us -- without this the whole cold fill ran at
            # half rate.  Dummy matmuls on zeroed SBUF depend on no DMA: they
            # run right out of the preamble (~7.5 us) and flip HAM to 2.4 GHz
            # by ~11 us.  Their PSUM target is slot 0 of the p00 ring, whose
            # first real user (q=1) starts long after they retire.
            warm_x = cst.tile([128, 128], X_DT)
            warm_w = cst.tile([128, NB], W_DT)
            nc.vector.memzero(warm_x[:])
            nc.vector.memzero(warm_w[:])
            warm_p = ps.tile([128, NB], F32, tag="p00")
            for _ in range(9):
                nc.tensor.matmul(
                    warm_p[:], warm_x[:], warm_w[:], start=True, stop=True
                )
            # bias: 8 KB row on the (otherwise idle) SWDGE queue, replicated
            # to all partitions by gpsimd via POOL ports -- zero SDMA traffic.
            b2_sb = cst.tile([1, 2, D_HID], FP16)
            nc.gpsimd.dma_start(b2_sb[:], b2[:])
            brep_sb = cst.tile([128, 2, D_HID], FP16)
            nc.gpsimd.partition_broadcast(brep_sb[:], b2_sb[:])

            hw_eng = [nc.sync, nc.scalar]

            def load_x(q, h, eng):
                t = xp.tile([KP, KH, TQ], X_DT, tag=f"xq{q}h{h}")
                eng.dma_start(t[:], xQ[q, h])
                return t

            def load_w(nb, e, j, eng):
                t = wp.tile([KP, KQ, NB], W_DT, tag=f"w{e}q{j}")
                eng.dma_start(t[:], wP[nb, e, j])
                return t

            # Cold fill: the 5 MB that the q=0 groups consume, split evenly
            # across the two queues in exactly the order the (mi-interleaved,
            # expert-major) MM loop consumes it: xh0+waj0 first, then wa
            # quarters alternating with xh1, then wb quarters.  sC (16 KB)
            # trails; the first epilogue needs it at ~26 us.
            x_t = [[None, None] for _ in range(NQ)]
            w_cur = {}
            x_t[0][0] = load_x(0, 0, nc.sync)
            w_cur[0, 0] = load_w(0, 0, 0, nc.scalar)
            w_cur[0, 1] = load_w(0, 0, 1, nc.sync)
            x_t[0][1] = load_x(0, 1, nc.scalar)
            w_cur[0, 3] = load_w(0, 0, 3, nc.sync)
            w_cur[0, 2] = load_w(0, 0, 2, nc.scalar)
            w_cur[1, 1] = load_w(0, 1, 1, nc.sync)
            w_cur[1, 0] = load_w(0, 1, 0, nc.scalar)
            w_cur[1, 3] = load_w(0, 1, 3, nc.sync)
            w_cur[1, 2] = load_w(0, 1, 2, nc.scalar)
            sC_sb = cst.tile([128, MG, 2], F32)
            nc.sync.dma_start(sC_sb[:], sC[:])
            for q in range(1, NQ):
                x_t[q][0] = load_x(q, 0, hw_eng[q % 2])
                x_t[q][1] = load_x(q, 1, hw_eng[(q + 1) % 2])

            for nb in range(NBLK):
                nb_sl = bass.ts(nb, NB)
                w_next = {} if nb + 1 < NBLK else None
                for q in range(NQ):
                    # W(nb+1) prefetch: one 0.5 MB quarter per q slice.
                    if w_next is not None:
                        e, j = q // 4, q % 4
                        w_next[e, j] = load_w(nb + 1, e, j, hw_eng[q % 2])
                    # Both mi-groups interleaved per W quarter: the PE then
                    # consumes each arriving 0.5 MB quarter with 8 MMs
                    # (1.73 us >= the ~1.35 us arrival cadence), so the cold
                    # fill runs gap-free and HAM never re-throttles.  Expert-
                    # major: pa* stop 16 MMs early, so their epilogue ops
                    # overlap pb*'s accumulation.
                    pt = {
                        (e, mi): ps.tile(
                            [128, NB], F32, tag=f"p{e}{mi}", name=f"pt{e}{mi}"
                        )
                        for e in range(2)
                        for mi in range(MPQ)
                    }
                    for e in range(2):
                        for j in range(4):
                            # dummy heartbeats between the first group's
                            # quarter-blocks: when a quarter's DMA is late,
                            # these run in the gap and keep the HAM activity
                            # window busy (a single mid-fill re-throttle to
                            # 1.2 GHz costs ~6.5 us of wall time).
                            if nb == 0 and q == 0 and (e, j) != (0, 0):
                                for _ in range(2):
                                    nc.tensor.matmul(
                                        warm_p[:], warm_x[:], warm_w[:],
                                        start=True, stop=True,
                                    )
                            for mi in range(MPQ):
                                for kk in range(KQ):
                                    nc.tensor.matmul(
                                        pt[e, mi][:],
                                        x_t[q][j // 2][
                                            :, (j % 2) * KQ + kk, bass.ts(mi, 128)
                                        ],
                                        w_cur[e, j][:, kk, :],
                                        start=(j == 0 and kk == 0),
                                        stop=(j == 3 and kk == KQ - 1),
                                    )
                    for mi in range(MPQ):
                        mg = q * MPQ + mi
                        pa, pb = pt[0, mi], pt[1, mi]
                        s0 = sC_sb[:, mg, 0:1]
                        s1 = sC_sb[:, mg, 1:2]
                        # epilogue: out = s0*(pa+bA) + s1*(pb+bB).  u/t1 run
                        # during pb's accumulation (pa stopped 16 MMs ago);
                        # only v -> o trail the last MM.  t1 on ACT (per-
                        # partition scale AP) overlaps DVE's v add.
                        u = ep.tile([128, NB], F32, tag="u")
                        nc.vector.tensor_add(u[:], pa[:], brep_sb[:, 0, nb_sl])
                        t1 = ep.tile([128, NB], F32, tag="t1")
                        nc.scalar.activation(
                            t1[:], u[:], mybir.ActivationFunctionType.Copy,
                            scale=s0,
                        )
                        # the very last group's v->o->DMA chain is the kernel
                        # tail: split it into column halves on both queues.
                        nspl = 2 if (nb == NBLK - 1 and mg == MG - 1) else 1
                        hb = NB // nspl
                        v = ep.tile([128, NB], F32, tag="v")
                        o = op.tile([128, NB], O_DT, tag="o")
                        for c in range(nspl):
                            c_sl = bass.ts(c, hb)
                            oc_sl = slice(nb * NB + c * hb, nb * NB + (c + 1) * hb)
                            nc.vector.tensor_add(
                                v[:, c_sl], pb[:, c_sl],
                                brep_sb[:, 1, oc_sl],
                            )
                            nc.vector.scalar_tensor_tensor(
                                o[:, c_sl], v[:, c_sl], s1, t1[:, c_sl],
                                op0=MULT, op1=ADD,
                            )
                            hw_eng[(mg + c) % 2].dma_start(
                                out[bass.ts(mg, 128), oc_sl], o[:, c_sl]
                            )
                w_cur = w_next

    nc.compile()
    return nc


def _host_gating(x, W_gate, b_gate):
    logits = x @ W_gate + b_gate                       # [N, 8] fp32
    m = logits.max(axis=1, keepdims=True)
    e = np.exp(logits - m)
    gates = e / e.sum(axis=1, keepdims=True)
    idx0 = np.argsort(-gates[0], kind="stable")[:2]    # token-0 top-2 experts
    scores = -np.sort(-gates, axis=1)[:, :2]           # per-token top-2 values
    return idx0, np.ascontiguousarray(scores)


def kernel(x, W_experts, b_experts, W_gate, b_gate):
    global LAST_RESULT
    x = np.ascontiguousarray(np.asarray(x, dtype=np.float32))
    W_experts = np.asarray(W_experts, dtype=np.float32)
    b_experts = np.asarray(b_experts, dtype=np.float32)
    W_gate = np.asarray(W_gate, dtype=np.float32)
    b_gate = np.asarray(b_gate, dtype=np.float32)

    idx0, scores = _host_gating(x, W_gate, b_gate)
    w_np_dt = mybir.dt.np(W_DT)
    x_np_dt = mybir.dt.np(X_DT)
    # wP[nb, e, j, p, kk, n] = W_sel[e][(j*KQ+kk)*KP + p, nb*NB + n]
    w_sel = W_experts[idx0].astype(w_np_dt)            # [2, D_IN, D_HID]
    wP = np.ascontiguousarray(
        w_sel.reshape(2, 4, KQ, KP, NBLK, NB)          # e,j,kk,p,nb,n
        .transpose(4, 0, 1, 3, 2, 5)                   # nb,e,j,p,kk,n
    )
    b2 = np.ascontiguousarray(
        b_experts[idx0][None]                          # [1, 2, D_HID]
    ).astype(np.float16)

    xT_full = x.astype(x_np_dt).T                      # [D_IN, N]

    nc = _build()
    in_maps = []
    for c in range(N_CORES):
        sl = slice(c * NT, (c + 1) * NT)
        # xQ[q, h, p, kk, t] = x[c*NT + q*TQ + t, (h*KH+kk)*KP + p]
        xQ = np.ascontiguousarray(
            xT_full[:, sl].reshape(2, KH, KP, NQ, TQ).transpose(3, 0, 2, 1, 4)
        )
        in_maps.append(
            {
                "xQ": xQ,
                "wP": wP,
                "b2": b2,
                "sC": np.ascontiguousarray(
                    scores[sl].reshape(MG, 128, 2).transpose(1, 0, 2)
                ),
            }
        )

    res = run_bass_kernel_spmd(nc, in_maps, list(range(N_CORES)))
    LAST_RESULT = res
    return np.concatenate(
        [r["out"] for r in res.results], axis=0
    ).astype(np.float32)


# revision 26
# speedup vs baseline: 1.0087x; 1.0002x over previous
"""MoE layer (top-2 of 8 experts, selection shared across tokens) on 8 TRN2 cores.

Math (faithful to the reference):
    gates = softmax(x @ W_gate + b_gate)          [N, 8]
    idx0  = top-2 expert indices of token 0       [2]
    s     = per-token top-2 gate VALUES (desc)    [N, 2]
    out   = s0 * (x @ W[A] + b[A]) + s1 * (x @ W[B] + b[B])

Strategy: gating + top-2 is 0.2% of the FLOPs -> computed on host.  The two
active expert matmuls (275 GFLOP) are data-parallel sharded over tokens across
8 cores; expert weights are replicated.  Matmuls run in fp16 (values are small,
so fp16 range is safe and its 10-bit mantissa keeps rel-err ~3e-4),
accumulating fp32 in PSUM.

The PE stream (2048 MMs x 512 cols @ 2.4 GHz = 437 us) is the hard floor;
everything else is arranged to keep the PE saturated from ~11 us on:
  - x is resident in SBUF (loaded once, 8.4 MB fp16), W streams once per
    nb-block (no x re-streaming; total DMA-in 25 MB vs 50 MB).
  - DMA is coarse (0.5 MB tiles, >=4 KB per-partition descriptor runs):
    dma_start issue costs ~0.65 us, and SDMA round-robins between queues at
    PACKET granularity where a packet is one descriptor run -- small
    descriptors starve their queue.  x/W are host-PRE-PACKED per tile.
  - The cold fill is the binding constraint for the stream start: group 0
    needs xq0 + the full 4 MB W block, so those 5 MB are split evenly
    across both HWDGE queues in exactly the order the PE consumes them.
  - bias is replicated across partitions ON-CHIP (gpsimd partition_broadcast
    from an 8 KB row) so no constant competes with the fill; per-token
    scores (sC, 16 KB) trail the critical tiles.
  - outputs are written fp16 (host upcasts) on the HWDGE queues (the SWDGE
    end-of-kernel drain costs ~6.5 us; HWDGE drains instantly).
"""

import functools

import numpy as np

import concourse.bass as bass
import concourse.mybir as mybir
import concourse.tile as tile
from concourse import bacc
from concourse.bass_utils import run_bass_kernel_spmd

N_CORES = 8
N, D_IN, D_HID = 16384, 2048, 2048
NT = N // N_CORES            # tokens per core
KP = 128                     # contraction chunk = partition dim
KCH = D_IN // KP             # 16 K-chunks
KH = 8                       # K-chunks per x half-tile
KQ = 4                       # K-chunks per W quarter-tile
NB = 512                     # output column block (1 PSUM bank of fp32)
NBLK = D_HID // NB           # 4 output blocks
TQ = 256                     # tokens per resident x tile
NQ = NT // TQ                # 8 x tiles
MPQ = TQ // 128              # m-tiles per x tile
MG = NT // 128               # 16 token groups per core

F32 = mybir.dt.float32
FP16 = mybir.dt.float16

W_DT = FP16
X_DT = FP16
O_DT = FP16

# Filled by test harness inspection: last BassKernelResults from a run.
LAST_RESULT = None


@functools.lru_cache(maxsize=1)
def _build():
    nc = bacc.Bacc("TRN2", target_bir_lowering=False, debug=False)
    # Host-pre-packed: xQ[q, h] -> one x half-tile [128, KH, TQ] (4 KB/part);
    # wP[nb, e, j] -> one W quarter-tile [128, KQ, NB] (4 KB/part).
    xQ = nc.dram_tensor("xQ", [NQ, 2, 128, KH, TQ], X_DT, kind="ExternalInput")
    wP = nc.dram_tensor("wP", [NBLK, 2, 4, 128, KQ, NB], W_DT,
                        kind="ExternalInput")
    # bias row (replicated on-chip): b2[0, e, o] = b_sel[e, o]
    b2 = nc.dram_tensor("b2", [1, 2, D_HID], FP16, kind="ExternalInput")
    # per-token scores pre-arranged on host, partition-major:
    # sC[p, m, s] = top2_score[m*128 + p, s]
    sC = nc.dram_tensor("sC", [128, MG, 2], F32, kind="ExternalInput")
    out = nc.dram_tensor("out", [NT, D_HID], O_DT, kind="ExternalOutput")

    MULT = mybir.AluOpType.mult
    ADD = mybir.AluOpType.add

    with tile.TileContext(nc) as tc:
        with (
            tc.tile_pool(name="cst", bufs=1) as cst,
            tc.tile_pool(name="xp", bufs=1) as xp,
            tc.tile_pool(name="wp", bufs=2) as wp,
            tc.tile_pool(name="ep", bufs=2) as ep,
            tc.tile_pool(name="op", bufs=3) as op,
            tc.tile_pool(name="ps", bufs=2, space=bass.MemorySpace.PSUM) as ps,
        ):
            # PE warm-up: the HAM clock gate keeps the PE at 1.2 GHz until it
            # sees ~3.4 us of sustained activity, and the first real tiles
            # only land at ~10 us -- without this the whole cold fill ran at
            # half rate.  Dummy matmuls on zeroed SBUF depend on no DMA: they
            # run right out of the preamble (~7.5 us) and flip HAM to 2.4 GHz
            # by ~11 us.  Their PSUM target is slot 0 of the p00 ring, whose
            # first real user (q=1) starts long after they retire.
            warm_x = cst.tile([128, 128], X_DT)
            warm_w = cst.tile([128, NB], W_DT)
            nc.vector.memzero(warm_x[:])
            nc.vector.memzero(warm_w[:])
            warm_p = ps.tile([128, NB], F32, tag="p00")
            for _ in range(12):
                nc.tensor.matmul(
                    warm_p[:], warm_x[:], warm_w[:], start=True, stop=True
                )
            # bias: 8 KB row on the (otherwise idle) SWDGE queue, replicated
            # to all partitions by gpsimd via POOL ports -- zero SDMA traffic.
            b2_sb = cst.tile([1, 2, D_HID], FP16)
            nc.gpsimd.dma_start(b2_sb[:], b2[:])
            brep_sb = cst.tile([128, 2, D_HID], FP16)
            nc.gpsimd.partition_broadcast(brep_sb[:], b2_sb[:])

            hw_eng = [nc.sync, nc.scalar]

            def load_x(q, h, eng):
                t = xp.tile([KP, KH, TQ], X_DT, tag=f"xq{q}h{h}")
                eng.dma_start(t[:], xQ[q, h])
                return t

            def load_w(nb, e, j, eng):
                t = wp.tile([KP, KQ, NB], W_DT, tag=f"w{e}q{j}")
                eng.dma_start(t[:], wP[nb, e, j])
                return t

            # Cold fill: the 5 MB that the q=0 groups consume, split evenly
            # across the two queues in exactly the order the (mi-interleaved,
            # expert-major) MM loop consumes it: xh0+waj0 first, then wa
            # quarters alternating with xh1, then wb quarters.  sC (16 KB)
            # trails; the first epilogue needs it at ~26 us.
            x_t = [[None, None] for _ in range(NQ)]
            w_cur = {}
            x_t[0][0] = load_x(0, 0, nc.sync)
            w_cur[0, 0] = load_w(0, 0, 0, nc.scalar)
            w_cur[0, 1] = load_w(0, 0, 1, nc.sync)
            x_t[0][1] = load_x(0, 1, nc.scalar)
            w_cur[0, 3] = load_w(0, 0, 3, nc.sync)
            w_cur[0, 2] = load_w(0, 0, 2, nc.scalar)
            w_cur[1, 1] = load_w(0, 1, 1, nc.sync)
            w_cur[1, 0] = load_w(0, 1, 0, nc.scalar)
            w_cur[1, 3] = load_w(0, 1, 3, nc.sync)
            w_cur[1, 2] = load_w(0, 1, 2, nc.scalar)
            sC_sb = cst.tile([128, MG, 2], F32)
            nc.sync.dma_start(sC_sb[:], sC[:])
            for q in range(1, NQ):
                x_t[q][0] = load_x(q, 0, hw_eng[q % 2])
                x_t[q][1] = load_x(q, 1, hw_eng[(q + 1) % 2])

            for nb in range(NBLK):
                nb_sl = bass.ts(nb, NB)
                w_next = {} if nb + 1 < NBLK else None
                for q in range(NQ):
                    # W(nb+1) prefetch: one 0.5 MB quarter per q slice.
                    if w_next is not None:
                        e, j = q // 4, q % 4
                        w_next[e, j] = load_w(nb + 1, e, j, hw_eng[q % 2])
                    # Both mi-groups interleaved per W quarter: the PE then
                    # consumes each arriving 0.5 MB quarter with 8 MMs
                    # (1.73 us >= the ~1.35 us arrival cadence), so the cold
                    # fill runs gap-free and HAM never re-throttles.  Expert-
                    # major: pa* stop 16 MMs early, so their epilogue ops
                    # overlap pb*'s accumulation.
                    pt = {
                        (e, mi): ps.tile(
                            [128, NB], F32, tag=f"p{e}{mi}", name=f"pt{e}{mi}"
                        )
                        for e in range(2)
                        for mi in range(MPQ)
                    }
                    for e in range(2):
                        for j in range(4):
                            # dummy heartbeats between the first group's
                            # quarter-blocks: when a quarter's DMA is late,
                            # these run in the gap and keep the HAM activity
                            # window busy (a single mid-fill re-throttle to
                            # 1.2 GHz costs ~6.5 us of wall time).
                            if nb == 0 and q == 0 and (e, j) != (0, 0):
                                for _ in range(2):
                                    nc.tensor.matmul(
                                        warm_p[:], warm_x[:], warm_w[:],
                                        start=True, stop=True,
                                    )
                            for mi in range(MPQ):
                                for kk in range(KQ):
                                    nc.tensor.matmul(
                                        pt[e, mi][:],
                                        x_t[q][j // 2][
                                            :, (j % 2) * KQ + kk, bass.ts(mi, 128)
                                        ],
                                        w_cur[e, j][:, kk, :],
                                        start=(j == 0 and kk == 0),
                                        stop=(j == 3 and kk == KQ - 1),
                                    )
                    for mi in range(MPQ):
                        mg = q * MPQ + mi
                        pa, pb = pt[0, mi], pt[1, mi]
                        s0 = sC_sb[:, mg, 0:1]
                        s1 = sC_sb[:, mg, 1:2]
                        # epilogue: out = s0*(pa+bA) + s1*(pb+bB).  u/t1 run
                        # during pb's accumulation (pa stopped 16 MMs ago);
                        # only v -> o trail the last MM.  t1 on ACT (per-
                        # partition scale AP) overlaps DVE's v add.
                        u = ep.tile([128, NB], F32, tag="u")
                        nc.vector.tensor_add(u[:], pa[:], brep_sb[:, 0, nb_sl])
                        t1 = ep.tile([128, NB], F32, tag="t1")
                        nc.scalar.activation(
                            t1[:], u[:], mybir.ActivationFunctionType.Copy,
                            scale=s0,
                        )
                        # the very last group's v->o->DMA chain is the kernel
                        # tail: split it into column halves on both queues.
                        nspl = 2 if (nb == NBLK - 1 and mg == MG - 1) else 1
                        hb = NB // nspl
                        v = ep.tile([128, NB], F32, tag="v")
                        o = op.tile([128, NB], O_DT, tag="o")
                        for c in range(nspl):
                            c_sl = bass.ts(c, hb)
                            oc_sl = slice(nb * NB + c * hb, nb * NB + (c + 1) * hb)
                            nc.vector.tensor_add(
                                v[:, c_sl], pb[:, c_sl],
                                brep_sb[:, 1, oc_sl],
                            )
                            nc.vector.scalar_tensor_tensor(
                                o[:, c_sl], v[:, c_sl], s1, t1[:, c_sl],
                                op0=MULT, op1=ADD,
                            )
                            hw_eng[(mg + c) % 2].dma_start(
                                out[bass.ts(mg, 128), oc_sl], o[:, c_sl]
                            )
                w_cur = w_next

    nc.compile()
    return nc


def _host_gating(x, W_gate, b_gate):
    logits = x @ W_gate + b_gate                       # [N, 8] fp32
    m = logits.max(axis=1, keepdims=True)
    e = np.exp(logits - m)
    gates = e / e.sum(axis=1, keepdims=True)
    idx0 = np.argsort(-gates[0], kind="stable")[:2]    # token-0 top-2 experts
    scores = -np.sort(-gates, axis=1)[:, :2]           # per-token top-2 values
    return idx0, np.ascontiguousarray(scores)


def kernel(x, W_experts, b_experts, W_gate, b_gate):
    global LAST_RESULT
    x = np.ascontiguousarray(np.asarray(x, dtype=np.float32))
    W_experts = np.asarray(W_experts, dtype=np.float32)
    b_experts = np.asarray(b_experts, dtype=np.float32)
    W_gate = np.asarray(W_gate, dtype=np.float32)
    b_gate = np.asarray(b_gate, dtype=np.float32)

    idx0, scores = _host_gating(x, W_gate, b_gate)
    w_np_dt = mybir.dt.np(W_DT)
    x_np_dt = mybir.dt.np(X_DT)
    # wP[nb, e, j, p, kk, n] = W_sel[e][(j*KQ+kk)*KP + p, nb*NB + n]
    w_sel = W_experts[idx0].astype(w_np_dt)            # [2, D_IN, D_HID]
    wP = np.ascontiguousarray(
        w_sel.reshape(2, 4, KQ, KP, NBLK, NB)          # e,j,kk,p,nb,n
        .transpose(4, 0, 1, 3, 2, 5)                   # nb,e,j,p,kk,n
    )
    b2 = np.ascontiguousarray(
        b_experts[idx0][None]                          # [1, 2, D_HID]
    ).astype(np.float16)

    xT_full = x.astype(x_np_dt).T                      # [D_IN, N]

    nc = _build()
    in_maps = []
    for c in range(N_CORES):
        sl = slice(c * NT, (c + 1) * NT)
        # xQ[q, h, p, kk, t] = x[c*NT + q*TQ + t, (h*KH+kk)*KP + p]
        xQ = np.ascontiguousarray(
            xT_full[:, sl].reshape(2, KH, KP, NQ, TQ).transpose(3, 0, 2, 1, 4)
        )
        in_maps.append(
            {
                "xQ": xQ,
                "wP": wP,
                "b2": b2,
                "sC": np.ascontiguousarray(
                    scores[sl].reshape(MG, 128, 2).transpose(1, 0, 2)
                ),
            }
        )

    res = run_bass_kernel_spmd(nc, in_maps, list(range(N_CORES)))
    LAST_RESULT = res
    return np.concatenate(
        [r["out"] for r in res.results], axis=0
    ).astype(np.float32)
